# revision 51
# baseline (speedup 1.0000x reference)
"""Bass/Trainium2 kernel for nn_Decoder: oscillator-bank waveform decoder.

out[m, w] = mean_k( amp[m,k] * cos(phi[m,k] + pi*z_k*w) )
  phi = pi*(x @ W_arg.T + b_arg),  amp = x @ W_amp.T + b_amp

Data-parallel over the batch*length dim across 8 NeuronCores; parameters
(the two 128x1024 linears, biases, and the 128x2048 cos/sin phase tables)
are replicated. All matmuls run in fp16 with fp32 PSUM accumulation.

Per-core pipeline (M_c = 1024 rows, four 256-row blocks):
  GEMM1: phiT/ampT[k_osc=128, m]  = W_T.T @ x_T               (PE)
  trig:  mod-2 range reduction via magic-number round (DVE),
         y=v+g on GpSimd, Sin LUT + amp bias on ACT
  GEMM2: out[m, w] = cT.T @ costab + sT.T @ sintab            (PE)
  psum -> sbuf fp16 copies alternate DVE/ACT, batched DMA out.

DMAs are batched via multi-dim access patterns (one DMA per logical
tensor) because each dma_start costs ~0.65us of sequencer issue time.
"""

import functools

import numpy as np

import concourse.mybir as mybir
from concourse import bacc
from concourse.tile import TileContext
from concourse.bass_utils import run_bass_kernel_spmd

# Pin every ACT function we use (Sin/Identity/Copy) to the single
# 'trig_and_small' table set: the default per-activation chooser picks the
# first set containing the function, thrashing LoadActFuncSet (~1.3us each)
# between the copy/identity set and the sin set. Removing those functions
# from all other sets (names and ids unchanged) forces one load total.
_orig_get_act_tables = bacc.get_activation_tables


@functools.cache
def _pinned_act_tables(module_arch):
    full = dict(_orig_get_act_tables(module_arch))
    pin = {
        mybir.ActivationFunctionType.Sin,
        mybir.ActivationFunctionType.Identity,
        mybir.ActivationFunctionType.Copy,
    }
    out = {}
    for name, funcs in full.items():
        if name == "trig_and_small":
            out[name] = funcs
        else:
            out[name] = funcs - pin
    return out


bacc.get_activation_tables = _pinned_act_tables

PI = float(np.pi)
MAGIC = 1.5 * 2.0**23  # fp32 round-to-nearest-int magic constant

BATCH, LENGTH, DIM = 16, 512, 1024
K = 128           # oscillators
WLEN = 2048       # waveform length
NCORES = 8
M_TOTAL = BATCH * LENGTH          # 8192
MC = M_TOTAL // NCORES            # 1024 rows per core
MB = 256                          # m-block (GEMM1 free dim)
NMB = MC // MB                    # 4
KCH = DIM // 128                  # 8 contraction chunks

F32 = mybir.dt.float32
F16 = mybir.dt.float16

_cached_nc = None


def _build():
    nc = bacc.Bacc("TRN2", target_bir_lowering=False, debug=False)

    xT = nc.dram_tensor("xT", [DIM, MC], F16, kind="ExternalInput")
    wtab = nc.dram_tensor("wtab", [DIM, 2 * K], F16, kind="ExternalInput")
    bias = nc.dram_tensor("bias", [K, 3], F32, kind="ExternalInput")
    tabs = nc.dram_tensor("tabs", [K, 2 * WLEN], F16, kind="ExternalInput")
    out = nc.dram_tensor("out", [MC, WLEN], F16, kind="ExternalOutput")

    with TileContext(nc) as tc:
        with (
            tc.tile_pool(name="consts", bufs=1) as cpool,
            tc.tile_pool(name="xp", bufs=3) as xpool,
            tc.tile_pool(name="trig", bufs=2) as tpool,
            tc.tile_pool(name="cs", bufs=2) as cspool,
            tc.tile_pool(name="ob", bufs=4) as opool,
            tc.tile_pool(name="ps1", bufs=1, space="PSUM") as ps1,
            tc.tile_pool(name="ps2", bufs=3, space="PSUM") as ps2,
        ):
            # --- replicated parameters -> SBUF, one batched DMA each ---
            # weights: DRAM [1024=(k p), 256=(arg|amp)] -> [128, (k 256)]
            wt = cpool.tile([128, KCH * 2 * K], F16, tag="wt")
            nc.sync.dma_start(
                wt[:, :].rearrange("p (k j) -> p k j", k=KCH),
                wtab[:, :].rearrange("(k p) j -> p k j", p=128))
            bias_t = cpool.tile([128, 3], F32, tag="bias")
            nc.gpsimd.dma_start(bias_t[:, :], bias[:, :])
            tabw = []
            for w in range(4):
                tabw_tile = cpool.tile([128, 1024], F16, tag=f"tabw{w}")
                tabw.append(tabw_tile)
            bcos_t = bias_t[:, 0:1]
            bsin_t = bias_t[:, 1:2]
            bamp_t = bias_t[:, 2:3]

            # PE warmup: ~4us of dummy matmuls on scratch SBUF so the HAM
            # clock-gate releases (1.2 -> 2.4 GHz) before the first real
            # matmul; they have no DMA dependency so they start immediately.
            scratch = cpool.tile([128, 128], F16, tag="scratch")
            nc.vector.memset(scratch[:, :], 0.0)
            warm_ps = ps2.tile([128, 1024], F32, tag="ops")
            for _ in range(68):
                nc.tensor.matmul(
                    warm_ps[:, 0:64], scratch[:, :], scratch[:, 0:64],
                    start=True, stop=True)

            def warg(k):
                return wt[:, k * 2 * K:k * 2 * K + K]

            def wamp(k):
                return wt[:, k * 2 * K + K:(k + 1) * 2 * K]

            def gemm1(mb):
                m0 = mb * MB
                xt = xpool.tile([128, KCH * MB], F16, tag="xt")
                nsplit = 2
                kc = KCH // nsplit
                for h in range(nsplit):
                    nc.sync.dma_start(
                        xt[:, h * kc * MB:(h + 1) * kc * MB].rearrange(
                            "p (k m) -> p k m", k=kc),
                        xT[h * kc * 128:(h + 1) * kc * 128,
                           m0:m0 + MB].rearrange("(k p) m -> p k m", p=128))
                phi_ps = ps1.tile([128, MB], F32, tag="phi")
                amp_ps = ps1.tile([128, MB], F32, tag="amp")
                for k in range(KCH):
                    nc.tensor.matmul(
                        phi_ps[:, :], warg(k), xt[:, k * MB:(k + 1) * MB],
                        start=(k == 0), stop=(k == KCH - 1))
                for k in range(KCH):
                    nc.tensor.matmul(
                        amp_ps[:, :], wamp(k), xt[:, k * MB:(k + 1) * MB],
                        start=(k == 0), stop=(k == KCH - 1))
                return phi_ps, amp_ps

            def trig(phi_ps, amp_ps):
                # cos branch: v = t + (b+0.5); y = v - 2*round(v/2); cos = sin(pi*y)
                # sin branch: v = t + b;       y = v - 2*round(v/2); -sin = sin(-pi*y)
                cT = cspool.tile([128, MB], F16, tag="cT")
                sT = cspool.tile([128, MB], F16, tag="sT")
                ampv = tpool.tile([128, MB], F32, tag="ampv")
                nc.scalar.activation(
                    ampv[:, :], amp_ps[:, :],
                    mybir.ActivationFunctionType.Identity, bias=bamp_t)
                for which, bias_t_, scale in (("c", bcos_t, PI), ("s", bsin_t, -PI)):
                    v = tpool.tile([128, MB], F32, tag=f"v{which}")
                    nc.vector.tensor_scalar_add(v[:, :], phi_ps[:, :], bias_t_)
                    h = tpool.tile([128, MB], F32, tag=f"h{which}")
                    nc.vector.tensor_scalar(
                        h[:, :], v[:, :], 0.5, MAGIC,
                        mybir.AluOpType.mult, mybir.AluOpType.add)
                    g = tpool.tile([128, MB], F32, tag=f"g{which}")
                    nc.vector.tensor_scalar(
                        g[:, :], h[:, :], MAGIC, -2.0,
                        mybir.AluOpType.subtract, mybir.AluOpType.mult)
                    y = tpool.tile([128, MB], F32, tag=f"y{which}")
                    nc.vector.tensor_add(y[:, :], v[:, :], g[:, :])
                    f = tpool.tile([128, MB], F32, tag=f"f{which}")
                    nc.scalar.activation(
                        f[:, :], y[:, :], mybir.ActivationFunctionType.Sin,
                        scale=scale)
                    dst = cT if which == "c" else sT
                    nc.gpsimd.tensor_mul(dst[:, :], ampv[:, :], f[:, :])
                return cT, sT

            def gemm2(mb, cT, sT):
                m0 = mb * MB
                for ms in range(MB // 128):
                    ob = opool.tile([128, WLEN], F16, tag="ob")
                    for half in range(2):
                        o_ps = ps2.tile([128, 1024], F32, tag="ops")
                        for wsub in range(2):
                            w = half * 2 + wsub
                            nc.tensor.matmul(
                                o_ps[:, wsub * 512:(wsub + 1) * 512],
                                cT[:, ms * 128:(ms + 1) * 128],
                                tabw[w][:, 0:512],
                                start=True, stop=False)
                            nc.tensor.matmul(
                                o_ps[:, wsub * 512:(wsub + 1) * 512],
                                sT[:, ms * 128:(ms + 1) * 128],
                                tabw[w][:, 512:1024],
                                start=False, stop=True)
                        dstv = ob[:, half * 1024:(half + 1) * 1024]
                        if half == 0:
                            nc.vector.tensor_copy(dstv, o_ps[:, :])
                        else:
                            nc.scalar.copy(dstv, o_ps[:, :])
                        nc.sync.dma_start(
                            out[m0 + ms * 128:m0 + (ms + 1) * 128,
                                half * 1024:(half + 1) * 1024], dstv)

            # pipeline: G1(0),G1(1),[tabs w0/w1],G1(2),[tabs w2/w3],
            #           trig0,G1(3),G2(0),trig1,G2(1),trig2,G2(2),trig3,G2(3)
            g1 = [gemm1(0), gemm1(1)]
            nc.sync.dma_start(tabw[0][:, :], tabs[:, 0:1024])
            nc.sync.dma_start(tabw[1][:, :], tabs[:, 1024:2048])
            g1.append(gemm1(2))
            nc.sync.dma_start(tabw[2][:, :], tabs[:, 2048:3072])
            nc.sync.dma_start(tabw[3][:, :], tabs[:, 3072:4096])
            cs = [trig(*g1[0])]
            if NMB > 3:
                g1.append(gemm1(3))
            for mb in range(NMB):
                if mb + 1 < NMB:
                    cs.append(trig(*g1[mb + 1]))
                gemm2(mb, *cs[mb])

    nc.finalize()
    return nc


def _get_nc():
    global _cached_nc
    if _cached_nc is None:
        _cached_nc = _build()
    return _cached_nc


def prep_in_maps(x, z_arg, W_arg, b_arg, W_amp, b_amp):
    x = np.asarray(x, dtype=np.float32)
    z_arg = np.asarray(z_arg, dtype=np.float32)
    W_arg = np.asarray(W_arg, dtype=np.float32)
    b_arg = np.asarray(b_arg, dtype=np.float32)
    W_amp = np.asarray(W_amp, dtype=np.float32)
    b_amp = np.asarray(b_amp, dtype=np.float32)

    xf = x.reshape(M_TOTAL, DIM)

    # packed replicated parameters (fp64 for the tables on host)
    wtab = np.concatenate([W_arg.T, W_amp.T], axis=1).astype(np.float16)
    bias = np.stack([b_arg + 0.5, b_arg, b_amp], axis=1).astype(np.float32)
    n = np.arange(WLEN, dtype=np.float64)
    phase = np.pi * z_arg.astype(np.float64)[:, None] * n[None, :]
    cost = (np.cos(phase) / K).astype(np.float16)
    sint = (np.sin(phase) / K).astype(np.float16)
    # per-w-chunk interleave: [cos_w | sin_w] blocks of 512 columns each
    tabs = np.concatenate(
        [np.concatenate([cost[:, w * 512:(w + 1) * 512],
                         sint[:, w * 512:(w + 1) * 512]], axis=1)
         for w in range(4)], axis=1)

    in_maps = []
    for c in range(NCORES):
        shard = xf[c * MC:(c + 1) * MC]                        # [MC, DIM]
        xT = np.ascontiguousarray(shard.T).astype(np.float16)  # [DIM, MC]
        in_maps.append({"xT": xT, "wtab": wtab, "bias": bias, "tabs": tabs})
    return in_maps


def kernel(x, z_arg, W_arg, b_arg, W_amp, b_amp, **run_kwargs):
    in_maps = prep_in_maps(x, z_arg, W_arg, b_arg, W_amp, b_amp)
    res = run_bass_kernel_spmd(
        _get_nc(), in_maps, core_ids=list(range(NCORES)), **run_kwargs)
    out = np.concatenate([r["out"] for r in res.results], axis=0)
    out = out.astype(np.float32).reshape(BATCH, LENGTH, WLEN)
    if run_kwargs:
        return out, res
    return out


# revision 83
# speedup vs baseline: 1.0250x; 1.0250x over previous
"""Bass/Trainium2 kernel for nn_Decoder: oscillator-bank waveform decoder.

out[m, w] = mean_k( amp[m,k] * cos(phi[m,k] + pi*z_k*w) )
  phi = pi*(x @ W_arg.T + b_arg),  amp = x @ W_amp.T + b_amp

Data-parallel over the batch*length dim across 8 NeuronCores; parameters
(the two 128x1024 linears, biases, and the 128x2048 cos/sin phase tables)
are replicated. All matmuls run in fp16 with fp32 PSUM accumulation.

Per-core pipeline (M_c = 1024 rows, four 256-row blocks):
  GEMM1: phiT/ampT[k_osc=128, m]  = W_T.T @ x_T               (PE)
  trig:  mod-2 range reduction via magic-number round (DVE),
         y=v+g on GpSimd, Sin LUT + amp bias on ACT
  GEMM2: out[m, w] = cT.T @ costab + sT.T @ sintab            (PE)
  psum -> sbuf fp16 copies alternate DVE/ACT, batched DMA out.

DMAs are batched via multi-dim access patterns (one DMA per logical
tensor) because each dma_start costs ~0.65us of sequencer issue time.
"""

import functools

import numpy as np

import concourse.mybir as mybir
from concourse import bacc
from concourse.tile import TileContext
from concourse.bass_utils import run_bass_kernel_spmd

# Pin every ACT function we use (Sin/Identity/Copy) to the single
# 'trig_and_small' table set: the default per-activation chooser picks the
# first set containing the function, thrashing LoadActFuncSet (~1.3us each)
# between the copy/identity set and the sin set. Removing those functions
# from all other sets (names and ids unchanged) forces one load total.
_orig_get_act_tables = bacc.get_activation_tables


@functools.cache
def _pinned_act_tables(module_arch):
    full = dict(_orig_get_act_tables(module_arch))
    pin = {
        mybir.ActivationFunctionType.Sin,
        mybir.ActivationFunctionType.Identity,
        mybir.ActivationFunctionType.Copy,
    }
    out = {}
    for name, funcs in full.items():
        if name == "trig_and_small":
            out[name] = funcs
        else:
            out[name] = funcs - pin
    return out


bacc.get_activation_tables = _pinned_act_tables

PI = float(np.pi)
MAGIC = 1.5 * 2.0**23  # fp32 round-to-nearest-int magic constant

BATCH, LENGTH, DIM = 16, 512, 1024
K = 128           # oscillators
WLEN = 2048       # waveform length
NCORES = 8
M_TOTAL = BATCH * LENGTH          # 8192
MC = M_TOTAL // NCORES            # 1024 rows per core
MBS = [256, 384, 256, 128]   # m-block sizes (sum = MC)
MOFF = [0, 256, 640, 896]    # block offsets
NMB = len(MBS)
MBMAX = max(MBS)
KCH = DIM // 128                  # 8 contraction chunks

F32 = mybir.dt.float32
F16 = mybir.dt.float16

_cached_nc = None


def _build():
    nc = bacc.Bacc("TRN2", target_bir_lowering=False, debug=False)

    xT = nc.dram_tensor("xT", [DIM, MC], F16, kind="ExternalInput")
    wtab = nc.dram_tensor("wtab", [DIM, 2 * K], F16, kind="ExternalInput")
    bias = nc.dram_tensor("bias", [K, 3], F32, kind="ExternalInput")
    tabs = nc.dram_tensor("tabs", [K, 2 * WLEN], F16, kind="ExternalInput")
    out = nc.dram_tensor("out", [MC, WLEN], F16, kind="ExternalOutput")

    with TileContext(nc) as tc:
        with (
            tc.tile_pool(name="consts", bufs=1) as cpool,
            tc.tile_pool(name="xp", bufs=3) as xpool,
            tc.tile_pool(name="trig", bufs=2) as tpool,
            tc.tile_pool(name="cs", bufs=2) as cspool,
            tc.tile_pool(name="ob", bufs=4) as opool,
            tc.tile_pool(name="ps1", bufs=1, space="PSUM") as ps1,
            tc.tile_pool(name="ps2", bufs=3, space="PSUM") as ps2,
        ):
            # --- replicated parameters -> SBUF, one batched DMA each ---
            # weights: DRAM [1024=(k p), 256=(arg|amp)] -> [128, (k 256)]
            wt = cpool.tile([128, KCH * 2 * K], F16, tag="wt")
            nc.sync.dma_start(
                wt[:, :].rearrange("p (k j) -> p k j", k=KCH),
                wtab[:, :].rearrange("(k p) j -> p k j", p=128))
            bias_t = cpool.tile([128, 3], F32, tag="bias")
            nc.gpsimd.dma_start(bias_t[:, :], bias[:, :])
            tabw = []
            for w in range(4):
                tabw_tile = cpool.tile([128, 1024], F16, tag=f"tabw{w}")
                tabw.append(tabw_tile)
            bcos_t = bias_t[:, 0:1]
            bsin_t = bias_t[:, 1:2]
            bamp_t = bias_t[:, 2:3]

            # PE warmup: ~4us of dummy matmuls on scratch SBUF so the HAM
            # clock-gate releases (1.2 -> 2.4 GHz) before the first real
            # matmul; they have no DMA dependency so they start immediately.
            scratch = cpool.tile([128, 128], F16, tag="scratch")
            nc.vector.memset(scratch[:, :], 0.0)
            warm_ps = ps2.tile([128, 1024], F32, tag="ops")
            for _ in range(68):
                nc.tensor.matmul(
                    warm_ps[:, 0:64], scratch[:, :], scratch[:, 0:64],
                    start=True, stop=True)

            def warg(k):
                return wt[:, k * 2 * K:k * 2 * K + K]

            def wamp(k):
                return wt[:, k * 2 * K + K:(k + 1) * 2 * K]

            def gemm1(mb):
                m0, MB = MOFF[mb], MBS[mb]
                xt = xpool.tile([128, KCH * MBMAX], F16, tag="xt")
                nsplit = 4 if mb == 0 else 2
                kc = KCH // nsplit
                for h in range(nsplit):
                    nc.sync.dma_start(
                        xt[:, h * kc * MB:(h + 1) * kc * MB].rearrange(
                            "p (k m) -> p k m", k=kc),
                        xT[h * kc * 128:(h + 1) * kc * 128,
                           m0:m0 + MB].rearrange("(k p) m -> p k m", p=128))
                phi_ps = ps1.tile([128, MBMAX], F32, tag="phi")
                amp_ps = ps1.tile([128, MBMAX], F32, tag="amp")
                for k in range(KCH):
                    nc.tensor.matmul(
                        phi_ps[:, 0:MB], warg(k), xt[:, k * MB:(k + 1) * MB],
                        start=(k == 0), stop=(k == KCH - 1))
                for k in range(KCH):
                    nc.tensor.matmul(
                        amp_ps[:, 0:MB], wamp(k), xt[:, k * MB:(k + 1) * MB],
                        start=(k == 0), stop=(k == KCH - 1))
                return mb, phi_ps, amp_ps

            def trig(mb, phi_ps, amp_ps):
                MB = MBS[mb]
                # cos branch: v = t + (b+0.5); y = v - 2*round(v/2); cos = sin(pi*y)
                # sin branch: v = t + b;       y = v - 2*round(v/2); -sin = sin(-pi*y)
                cT = cspool.tile([128, MBMAX], F16, tag="cT")
                sT = cspool.tile([128, MBMAX], F16, tag="sT")
                ampv = tpool.tile([128, MBMAX], F32, tag="ampv")
                nc.scalar.activation(
                    ampv[:, 0:MB], amp_ps[:, 0:MB],
                    mybir.ActivationFunctionType.Identity, bias=bamp_t)
                for which, bias_t_, scale in (("c", bcos_t, PI), ("s", bsin_t, -PI)):
                    v = tpool.tile([128, MBMAX], F32, tag=f"v{which}")
                    nc.vector.tensor_scalar_add(v[:, 0:MB], phi_ps[:, 0:MB], bias_t_)
                    h = tpool.tile([128, MBMAX], F32, tag=f"h{which}")
                    nc.vector.tensor_scalar(
                        h[:, 0:MB], v[:, 0:MB], 0.5, MAGIC,
                        mybir.AluOpType.mult, mybir.AluOpType.add)
                    g = tpool.tile([128, MBMAX], F32, tag=f"g{which}")
                    nc.vector.tensor_scalar(
                        g[:, 0:MB], h[:, 0:MB], MAGIC, -2.0,
                        mybir.AluOpType.subtract, mybir.AluOpType.mult)
                    y = tpool.tile([128, MBMAX], F32, tag=f"y{which}")
                    nc.vector.tensor_add(y[:, 0:MB], v[:, 0:MB], g[:, 0:MB])
                    f = tpool.tile([128, MBMAX], F32, tag=f"f{which}")
                    nc.scalar.activation(
                        f[:, 0:MB], y[:, 0:MB], mybir.ActivationFunctionType.Sin,
                        scale=scale)
                    dst = cT if which == "c" else sT
                    nc.gpsimd.tensor_mul(dst[:, 0:MB], ampv[:, 0:MB], f[:, 0:MB])
                return cT, sT

            def gemm2(mb, cT, sT):
                m0, MB = MOFF[mb], MBS[mb]
                for ms in range(MB // 128):
                    ob = opool.tile([128, WLEN], F16, tag="ob")
                    for half in range(2):
                        o_ps = ps2.tile([128, 1024], F32, tag="ops")
                        for wsub in range(2):
                            w = half * 2 + wsub
                            nc.tensor.matmul(
                                o_ps[:, wsub * 512:(wsub + 1) * 512],
                                cT[:, ms * 128:(ms + 1) * 128],
                                tabw[w][:, 0:512],
                                start=True, stop=False)
                            nc.tensor.matmul(
                                o_ps[:, wsub * 512:(wsub + 1) * 512],
                                sT[:, ms * 128:(ms + 1) * 128],
                                tabw[w][:, 512:1024],
                                start=False, stop=True)
                        dstv = ob[:, half * 1024:(half + 1) * 1024]
                        if half == 0:
                            nc.vector.tensor_copy(dstv, o_ps[:, :])
                        else:
                            nc.scalar.copy(dstv, o_ps[:, :])
                        nc.sync.dma_start(
                            out[m0 + ms * 128:m0 + (ms + 1) * 128,
                                half * 1024:(half + 1) * 1024], dstv)

            # pipeline: G1(0),G1(1),[tabs w0/w1],G1(2),[tabs w2/w3],
            #           trig0,G1(3),G2(0),trig1,G2(1),trig2,G2(2),trig3,G2(3)
            g1 = [gemm1(0), gemm1(1)]
            nc.sync.dma_start(tabw[0][:, :], tabs[:, 0:1024])
            nc.sync.dma_start(tabw[1][:, :], tabs[:, 1024:2048])
            g1.append(gemm1(2))
            nc.sync.dma_start(tabw[2][:, :], tabs[:, 2048:3072])
            nc.sync.dma_start(tabw[3][:, :], tabs[:, 3072:4096])
            cs = [trig(*g1[0])]
            for mb in range(3, NMB):
                g1.append(gemm1(mb))
            for mb in range(NMB):
                if mb + 1 < NMB:
                    cs.append(trig(*g1[mb + 1]))
                gemm2(mb, *cs[mb])

    nc.finalize()
    return nc


def _get_nc():
    global _cached_nc
    if _cached_nc is None:
        _cached_nc = _build()
    return _cached_nc


def prep_in_maps(x, z_arg, W_arg, b_arg, W_amp, b_amp):
    x = np.asarray(x, dtype=np.float32)
    z_arg = np.asarray(z_arg, dtype=np.float32)
    W_arg = np.asarray(W_arg, dtype=np.float32)
    b_arg = np.asarray(b_arg, dtype=np.float32)
    W_amp = np.asarray(W_amp, dtype=np.float32)
    b_amp = np.asarray(b_amp, dtype=np.float32)

    xf = x.reshape(M_TOTAL, DIM)

    # packed replicated parameters (fp64 for the tables on host)
    wtab = np.concatenate([W_arg.T, W_amp.T], axis=1).astype(np.float16)
    bias = np.stack([b_arg + 0.5, b_arg, b_amp], axis=1).astype(np.float32)
    n = np.arange(WLEN, dtype=np.float64)
    phase = np.pi * z_arg.astype(np.float64)[:, None] * n[None, :]
    cost = (np.cos(phase) / K).astype(np.float16)
    sint = (np.sin(phase) / K).astype(np.float16)
    # per-w-chunk interleave: [cos_w | sin_w] blocks of 512 columns each
    tabs = np.concatenate(
        [np.concatenate([cost[:, w * 512:(w + 1) * 512],
                         sint[:, w * 512:(w + 1) * 512]], axis=1)
         for w in range(4)], axis=1)

    in_maps = []
    for c in range(NCORES):
        shard = xf[c * MC:(c + 1) * MC]                        # [MC, DIM]
        xT = np.ascontiguousarray(shard.T).astype(np.float16)  # [DIM, MC]
        in_maps.append({"xT": xT, "wtab": wtab, "bias": bias, "tabs": tabs})
    return in_maps


def kernel(x, z_arg, W_arg, b_arg, W_amp, b_amp, **run_kwargs):
    in_maps = prep_in_maps(x, z_arg, W_arg, b_arg, W_amp, b_amp)
    res = run_bass_kernel_spmd(
        _get_nc(), in_maps, core_ids=list(range(NCORES)), **run_kwargs)
    out = np.concatenate([r["out"] for r in res.results], axis=0)
    out = out.astype(np.float32).reshape(BATCH, LENGTH, WLEN)
    if run_kwargs:
        return out, res
    return out
